# revision 12
# baseline (speedup 1.0000x reference)
"""Masked mean-pool (NonZeroAvgPool) Trainium2 Bass kernel.

out[b, d] = sum_s (tokens[b,s] != 0) * x[b,s,d] / sum_s (tokens[b,s] != 0)

Full shapes: x [16, 4096, 512] f32, tokens [16, 4096] i32 -> out [16, 512] f32.
Sharding: pure data parallel over batch; 2 batches per core on 8 cores.

Per-core program (shapes [2, 4096, 512] / [2, 4096] -> [2, 512]):
  - sequence rows are indexed s = p*32 + c  (p: SBUF partition, c: chunk)
    so every DMA is contiguous per partition.
  - valid[p, c] = (tokens != 0) as f32 via DVE not_equal
  - count      = ones[128,1].T @ rowsum(valid)         (PE, [1,1] PSUM)
  - num[1, D]  = sum_c valid[:, c].T @ x_tile[:, c, :] (PE, PSUM accumulate)
  - out row    = num * (1/count)  (DVE+ACT halves), then a 2KB store per
    batch (b0's store overlaps b1's stream).

DMA plan (8 dma_starts total - hard-won constraints from HW testing):
  - >8 DMAs reuses the 8 DMAHW completion-sem lanes; it usually works but
    crashed ~1-in-5 runs with NRT_EXEC_UNIT_UNRECOVERABLE.  Stay at 8.
  - The descriptor spray over the 16 SDMA engines splits a DMA's outer
    (partition) AP dim by its largest power-of-2 factor, onto engines
    [0, k): 128 parts -> all 16 engines; 124 parts -> engines 0-3 only
    (measured).  Per-descriptor engine overhead ~15ns, so bigger
    descriptors win.
  - SDMA engine 15 ran ~19% slower than the rest in 5 of 6 profiled runs
    (cause external to the kernel).  Every 128-partition DMA gives it
    exactly 1/16 of the bytes, so it paces the whole stream.

So: chunks 29-31 (~9% of bytes, both batches) load via two 8-SPRAY DMAs
([120, 2b, 3c, D] and [8, 2b, 3c, D] -> engines 0-7 only), queued FIRST.
Chunks 0-28 stream 128-partition/16-spray: batch 0 as ONE DMA (64KB-class
descriptors; its 32-matmul burst, including cold-HAM warmup, hides under
batch 1's stream), batch 1 tapered (17, 9, 3) so its matmuls track the
stream and the last semaphore gates a 3-chunk burst.  Engine 15 then
carries 15/16 of a normal share while engines 0-7 carry 17/16: with a
slow engine 15 the two sides finish together (~44us stream); with a
healthy one the cost is ~3.5us.  Either way exec lands ~58us instead of
63-68 on straggler runs.

Matmul chains run chunks 29-31 first (PSUM accumulation is order-free),
so the early-landing 8-spray data is consumed early and the taper's last
semaphore stays the only thing on the tail: 3 matmuls + divide + store.
"""

import os
from contextlib import ExitStack

import numpy as np

import concourse.bacc as bacc
import concourse.bass as bass
import concourse.tile as tile
from concourse import mybir
from concourse.bass_utils import run_bass_kernel_spmd

B, S, D = 16, 4096, 512
NCORES = 8
BPC = B // NCORES  # batches per core = 2
P = 128            # SBUF partitions
CPB = S // P       # chunks per batch = 32

# Chunks moved to the 8-spray (engines 0-7) side: the last XC chunks.
XC = int(os.environ.get("K_XC", "3"))
CM = CPB - XC      # chunks carried by the 16-spray main stream
# Main-stream chunks-per-dma_start per batch (over chunks [0, CM)).
GROUPS = [
    [int(g) for g in part.split(",")]
    for part in os.environ.get("K_GROUPS", "29/17,9,3").split("/")
]
assert all(sum(gs) == CM for gs in GROUPS) and len(GROUPS) == BPC
X_ENGINE = os.environ.get("K_XENG", "act")  # sync | act | gpsimd
PSPLIT = 120       # 8-spray partition split: [0,120) + [120,128)

_NC = None


def _build_nc():
    # Bacc (not plain Bass): its compile() runs generate_event_semaphores,
    # which splits multi-wait instructions onto InstEventSemaphore — TRN2
    # instructions can carry at most one sem wait.
    nc = bacc.Bacc(trn_type="TRN2")
    x = nc.dram_tensor("x", [BPC, S, D], mybir.dt.float32, kind="ExternalInput")
    tokens = nc.dram_tensor("tokens", [BPC, S], mybir.dt.int32, kind="ExternalInput")
    out = nc.dram_tensor("out", [BPC, D], mybir.dt.float32, kind="ExternalOutput")

    # s = p*CPB + c : per-partition contiguous rows
    xa = x[:].rearrange("b (p c) d -> p b c d", p=P)   # [128, BPC, 32, 512]
    ta = tokens[:].rearrange("b (p c) -> p b c", p=P)  # [128, BPC, 32]
    oa = out[:].rearrange("b d -> (b d)")              # [BPC*512]

    with TileKernel(nc) as tk:
        tk.body(xa, ta, oa)
    nc.compile()
    return nc


class TileKernel:
    def __init__(self, nc):
        self.nc = nc
        self.ctx = ExitStack()
        self.tc = None

    def __enter__(self):
        self.tc = self.ctx.enter_context(tile.TileContext(self.nc))
        return self

    def __exit__(self, *exc):
        return self.ctx.__exit__(*exc)

    def body(self, xa, ta, oa):
        nc = self.nc
        tc = self.tc
        ctx = self.ctx

        xpool = ctx.enter_context(tc.tile_pool(name="xpool", bufs=1))
        vpool = ctx.enter_context(tc.tile_pool(name="vpool", bufs=1))
        spool = ctx.enter_context(tc.tile_pool(name="spool", bufs=2))
        singles = ctx.enter_context(tc.tile_pool(name="singles", bufs=1))
        psum = ctx.enter_context(tc.tile_pool(name="psum", bufs=2, space="PSUM"))

        xeng = {"sync": nc.sync, "act": nc.scalar, "gpsimd": nc.gpsimd}[X_ENGINE]

        # One static x tile; every DMA writes its own region exactly once
        # (no ring, no WAR hazards).  float32r: the DMA is a pure bit copy;
        # single-pass fp32 matmul (4x faster than fp32's two half-rate
        # passes); mask weights are exact 0/1, PSUM accumulates in fp32.
        xbig = xpool.tile([P, BPC, CPB, D], mybir.dt.float32r, name="xbig")
        xr = xa.bitcast(mybir.dt.float32r)

        # 8-spray extras first: engines 0-7 start on them while engines
        # 8-15 go straight to the main stream.
        if XC:
            xeng.dma_start(
                out=xbig[:PSPLIT, :, CM:, :], in_=xr[:PSPLIT, :, CM:, :]
            )
            xeng.dma_start(
                out=xbig[PSPLIT:, :, CM:, :], in_=xr[PSPLIT:, :, CM:, :]
            )
        # 16-spray main stream: all of b0, then b1's taper.
        for b in range(BPC):
            c0 = 0
            for g in GROUPS[b]:
                xeng.dma_start(
                    out=xbig[:, b, c0:c0 + g, :], in_=xr[:, b, c0:c0 + g, :]
                )
                c0 += g

        # --- mask + counts for both batches (one tok DMA) --------------------
        tok = vpool.tile([P, BPC, CPB], mybir.dt.int32)
        nc.sync.dma_start(out=tok, in_=ta)
        # valid is declared float32r so the fp32r matmul's verifier sees a
        # rounded producer; its values (0.0/1.0) are exact in any precision.
        valid = vpool.tile([P, BPC, CPB], mybir.dt.float32r)
        nc.vector.tensor_scalar(
            out=valid, in0=tok, scalar1=0, scalar2=None,
            op0=mybir.AluOpType.not_equal,
        )
        rowsum = spool.tile([P, BPC], mybir.dt.float32)
        nc.vector.reduce_sum(
            out=rowsum, in_=valid.bitcast(mybir.dt.float32),
            axis=mybir.AxisListType.X,
        )

        ones = singles.tile([P, 1], mybir.dt.float32)
        nc.vector.memset(ones, 1.0)

        orow = [
            spool.tile([1, D], mybir.dt.float32, name=f"orow{b}")
            for b in range(BPC)
        ]

        for b in range(BPC):
            cnt = psum.tile([1, 1], mybir.dt.float32)
            nc.tensor.matmul(cnt, ones, rowsum[:, b:b + 1], start=True, stop=True)
            recip = spool.tile([1, 1], mybir.dt.float32)
            nc.vector.reciprocal(recip, cnt)

            # --- masked sum: extras' chunks first (their data lands first;
            # PSUM accumulation order is irrelevant), then the main chunks.
            num = psum.tile([1, D], mybir.dt.float32)
            order = list(range(CM, CPB)) + list(range(CM))
            for i, c in enumerate(order):
                nc.tensor.matmul(
                    num, valid[:, b, c:c + 1], xbig[:, b, c, :],
                    start=(i == 0), stop=(i == CPB - 1),
                )

            # --- divide + store: split across DVE and ACT so the halves run
            # in parallel; only b1's store sits on the tail.
            h = D // 2
            nc.vector.tensor_scalar_mul(orow[b][:, :h], num[:, :h], recip)
            nc.scalar.mul(orow[b][:, h:], num[:, h:], recip)
            nc.sync.dma_start(out=oa[b * D:(b + 1) * D], in_=orow[b])


def _get_nc():
    global _NC
    if _NC is None:
        _NC = _build_nc()
    return _NC


def _shard(x, tokens):
    x = np.ascontiguousarray(np.asarray(x, dtype=np.float32))
    tokens = np.ascontiguousarray(np.asarray(tokens, dtype=np.int32))
    return [
        {
            "x": x[c * BPC:(c + 1) * BPC],
            "tokens": tokens[c * BPC:(c + 1) * BPC],
        }
        for c in range(NCORES)
    ]


def kernel(x, tokens):
    res = run_bass_kernel_spmd(_get_nc(), _shard(x, tokens), core_ids=list(range(NCORES)))
    return np.concatenate([r["out"] for r in res.results], axis=0)


def _install_ntff_shim():
    """The agent image's antenv lacks axon_hooks, so bass_utils' trace path
    can't find the NTFF hook. Recreate the tiny get/set module and register
    trn_boot's ctypes-based hook against the injected libaxon_pjrt.so."""
    import sys
    import types

    if "antenv.axon_hooks" in sys.modules:
        return
    mod = types.ModuleType("antenv.axon_hooks")
    state = {"hook": None}
    mod.set_axon_ntff_profile_hook = lambda h: state.__setitem__("hook", h)
    mod.get_axon_ntff_profile_hook = lambda: state["hook"]
    sys.modules["antenv.axon_hooks"] = mod
    try:
        from trn_agent_boot.trn_boot import _ntff_profile_via_ctypes

        mod.set_axon_ntff_profile_hook(
            _ntff_profile_via_ctypes("/opt/axon/libaxon_pjrt.so")
        )
    except Exception:
        pass


def kernel_profiled(x, tokens):
    """Same as kernel() but with NTFF tracing; returns (out, BassKernelResults)."""
    _install_ntff_shim()
    res = run_bass_kernel_spmd(
        _get_nc(), _shard(x, tokens), core_ids=list(range(NCORES)), trace=True
    )
    out = np.concatenate([r["out"] for r in res.results], axis=0)
    return out, res
